# revision 27
# baseline (speedup 1.0000x reference)
"""GCN layer (X@W -> edge gather/scale -> segment-sum by dest -> +b -> relu)
as a Bass/Tile kernel on 8 Trainium2 NeuronCores.

Strategy (1D node partition, SPMD single program):
  - Nodes sharded 12500/core (destination shard).  Each core computes its
    XW shard with PE matmuls, then an AllGather replicates the full XW
    table [100352, 64] f32 into every core's DRAM.
  - Edges partitioned by destination shard, sorted by (dest block of 128,
    source bucket).  Sources are gathered from the table with dma_gather
    (int16 indices -> 4 source windows of 32768 table rows).
  - Per 128-edge chunk a selection matrix S[e, d] = val[e] * (dest[e]==d)
    is built in ONE DVE tensor_scalar op from a constant iota tile, then
    PE computes psum[128 dests, 64] += S^T @ G (gathered rows), giving the
    multiply + segment-sum in one matmul.  +bias and relu on eviction.

All chunk counts are padded to the max over cores so all 8 cores run the
same program (required for the collective / PJRT SPMD launch).
"""

import math
from contextlib import ExitStack

import numpy as np

import concourse.bacc as bacc
import concourse.mybir as mybir
import concourse.tile as tile
from concourse.bass import _add_dep_helper
from concourse.bass_utils import run_bass_kernel_spmd

# Problem constants (hardcoded per contract; kernel.py must be self-contained).
N = 100000
E = 1600000
FIN = 256
FOUT = 64
NCORES = 8

P = 128                      # partitions / block size
SHARD = N // NCORES          # 12500 dest nodes per core
NBLK = math.ceil(SHARD / P)  # 98 dest blocks per core
SHARD_PAD = NBLK * P         # 12544 (X zero-padded rows)
TABLE_ROWS = NCORES * SHARD_PAD  # 100352
WIN = 32768                  # int16-addressable source window (table rows)
NBUCKET = math.ceil(TABLE_ROWS / WIN)  # 4
SB_BLOCKS = 8                # dest blocks per super-batch
NSB = math.ceil(NBLK / SB_BLOCKS)
KH = FIN // P                # 2 contraction halves in the GEMM


def _build_plan(edge_row, edge_col, edge_vals):
    """Host-side edge partition/sort/pad.  Returns the uniform structure
    (shared across cores) + per-core staged arrays."""
    core = edge_row // SHARD
    r_local = edge_row - core * SHARD
    blk = r_local // P
    dest_in_blk = (r_local - blk * P).astype(np.float32)
    src_core = edge_col // SHARD
    table_row = src_core * SHARD_PAD + (edge_col - src_core * SHARD)
    bucket = table_row // WIN
    idx16 = (table_row - bucket * WIN).astype(np.int16)

    # sort edges by (core, blk, bucket)
    order = np.lexsort((bucket, blk, core))
    core_s = core[order]
    blk_s = blk[order]
    bucket_s = bucket[order]
    idx16_s = idx16[order]
    dest_s = dest_in_blk[order]
    val_s = edge_vals[order].astype(np.float32)

    # segment counts per (core, blk, bucket)
    seg_key = (core_s * NBLK + blk_s) * NBUCKET + bucket_s
    counts = np.bincount(seg_key, minlength=NCORES * NBLK * NBUCKET).reshape(
        NCORES, NBLK, NBUCKET
    )
    # uniform capacity (in chunks of 128 edges) per (blk, bucket): max over cores
    chunks_bb = np.ceil(counts / P).astype(np.int64).max(axis=0)  # [NBLK, NBUCKET]
    # guarantee at least one chunk per block overall (needed so PSUM gets reset)
    assert chunks_bb.sum(axis=1).min() >= 1
    cap_bb = chunks_bb * P

    # ---- static layout ----
    # stream order: (sb, bucket, blk in sb, chunk)
    sb_of_blk = np.arange(NBLK) // SB_BLOCKS
    # chunk columns per (sb, bucket): sum of chunks of its blocks
    # slot offsets for each (blk, bucket) within its (sb, bucket) stream
    slot_off = np.zeros((NBLK, NBUCKET), dtype=np.int64)
    sb_b_len = np.zeros((NSB, NBUCKET), dtype=np.int64)   # slots per (sb, bucket)
    for sb in range(NSB):
        blks = np.where(sb_of_blk == sb)[0]
        for b in range(NBUCKET):
            off = 0
            for bk in blks:
                slot_off[bk, b] = off
                off += cap_bb[bk, b]
            sb_b_len[sb, b] = off
    # global offsets: chunk columns and idx columns per (sb, bucket)
    chunk_col0 = np.zeros((NSB, NBUCKET), dtype=np.int64)
    idx_col0 = np.zeros((NSB, NBUCKET), dtype=np.int64)
    ccur = icur = 0
    for sb in range(NSB):
        for b in range(NBUCKET):
            chunk_col0[sb, b] = ccur
            idx_col0[sb, b] = icur
            ccur += sb_b_len[sb, b] // P
            icur += sb_b_len[sb, b] // 16
    CTOT = ccur   # total chunks per core
    ITOT = icur   # total idx columns per core

    # global slot index for every edge:
    #   slot = global_slot0[sb,b] + slot_off[blk,b] + rank_within_segment
    # where global_slot0 = chunk_col0 * 128
    first_of_seg = np.zeros(NCORES * NBLK * NBUCKET + 1, dtype=np.int64)
    np.cumsum(counts.reshape(-1), out=first_of_seg[1:])
    rank = np.arange(len(core_s)) - first_of_seg[seg_key]
    slot = (
        chunk_col0[sb_of_blk[blk_s], bucket_s] * P
        + slot_off[blk_s, bucket_s]
        + rank
    )

    # ---- per-core staged arrays ----
    idx_streams = np.zeros((NCORES, CTOT * P), dtype=np.int16)
    dest_streams = np.zeros((NCORES, CTOT * P), dtype=np.float32)
    val_streams = np.zeros((NCORES, CTOT * P), dtype=np.float32)
    for c in range(NCORES):
        m = core_s == c
        idx_streams[c, slot[m]] = idx16_s[m]
        dest_streams[c, slot[m]] = dest_s[m]
        val_streams[c, slot[m]] = val_s[m]

    # dest/val DRAM layout [128, CTOT]: chunk j, partition p <- stream[j*128+p]
    dest_np = dest_streams.reshape(NCORES, CTOT, P).transpose(0, 2, 1).copy()
    val_np = val_streams.reshape(NCORES, CTOT, P).transpose(0, 2, 1).copy()

    # idx DRAM layout [128, ITOT] int16: within each (sb,b) segment of the
    # stream, idx i -> partition i%16 (replicated over the 8 groups of 16),
    # column i//16
    idx_np = np.zeros((NCORES, P, ITOT), dtype=np.int16)
    for sb in range(NSB):
        for b in range(NBUCKET):
            L = int(sb_b_len[sb, b])
            if L == 0:
                continue
            s0 = int(chunk_col0[sb, b]) * P
            i0 = int(idx_col0[sb, b])
            seg = idx_streams[:, s0:s0 + L].reshape(NCORES, L // 16, 16)
            seg = seg.transpose(0, 2, 1)  # [NCORES, 16, L//16]
            idx_np[:, :, i0:i0 + L // 16] = np.tile(seg, (1, 8, 1))

    # per-block chunk list: (bucket, j_local_in_gather, global_chunk_col)
    blk_chunks = []
    for bk in range(NBLK):
        sb = int(sb_of_blk[bk])
        lst = []
        for b in range(NBUCKET):
            nch = int(chunks_bb[bk, b])
            j0 = int(slot_off[bk, b]) // P
            c0 = int(chunk_col0[sb, b]) + j0
            for k in range(nch):
                lst.append((b, j0 + k, c0 + k))
        blk_chunks.append(lst)

    struct = dict(
        chunks_bb=chunks_bb, sb_b_len=sb_b_len, chunk_col0=chunk_col0,
        idx_col0=idx_col0, CTOT=CTOT, ITOT=ITOT, blk_chunks=blk_chunks,
        sb_of_blk=sb_of_blk,
    )
    return struct, idx_np, dest_np, val_np


_NO_SPLIT = ("InstEventSemaphore", "InstDrain", "InstCollectiveCompute",
             "InstCall", "InstUnconditionalBranch", "InstConditionalBranch")


def _split_excess_waits(nc):
    """Deterministic post-pass: TRN2 instructions tolerate very few sync
    waits (walrus rejects with 'Too many sync wait commands'; Bacc's own
    generate_event_semaphores pass misses cases).  Move all but one
    semaphore wait of every ordinary instruction onto wait-only
    InstEventSemaphore instructions inserted just before it on the same
    engine (engine program order then gates the original instruction)."""
    import concourse.mybir as mybir

    for blk in nc.main_func.blocks:
        out = []
        for ins in blk.instructions:
            si = ins.sync_info
            tn = type(ins).__name__
            if si is None or tn in _NO_SPLIT or len(si.on_wait) <= 1:
                out.append(ins)
                continue
            waits = list(si.on_wait)
            keep, excess = waits[:1], waits[1:]
            while excess:
                batch, excess = excess[:2], excess[2:]
                ev = mybir.InstEventSemaphore(
                    name=nc.get_next_instruction_name(), ins=[], outs=[])
                ev.engine = ins.engine
                ev.sync_info = mybir.SyncInfo(on_wait=batch, on_update=[])
                out.append(ev)
            ins.sync_info = mybir.SyncInfo(
                on_wait=keep, on_update=list(si.on_update))
            out.append(ins)
        blk.instructions[:] = out


def _build_nc(struct, variant="full"):
    # variant: "full" | "p12" (GEMM+collective, dummy out) |
    #          "p12g" (+ gathers, dummy out)
    st = struct
    CTOT, ITOT = st["CTOT"], st["ITOT"]
    nc = bacc.Bacc("TRN2", target_bir_lowering=False, debug=False,
                   num_devices=NCORES)
    f32 = mybir.dt.float32
    i16 = mybir.dt.int16

    xt_sh = nc.dram_tensor("xt_sh", [FIN, SHARD_PAD], f32, kind="ExternalInput")
    w_in = nc.dram_tensor("w_in", [FIN, FOUT], f32, kind="ExternalInput")
    b_rep = nc.dram_tensor("b_rep", [P, FOUT], f32, kind="ExternalInput")
    iota_in = nc.dram_tensor("iota_in", [P, P], f32, kind="ExternalInput")
    idx_in = nc.dram_tensor("idx_in", [P, ITOT], i16, kind="ExternalInput")
    dest_in = nc.dram_tensor("dest_in", [P, CTOT], f32, kind="ExternalInput")
    val_in = nc.dram_tensor("val_in", [P, CTOT], f32, kind="ExternalInput")

    xw_sh = nc.dram_tensor("xw_sh", [SHARD_PAD, FOUT], f32, kind="Internal")
    table = nc.dram_tensor("table", [TABLE_ROWS, FOUT], f32, kind="Internal",
                           addr_space="Shared")
    out_sh = nc.dram_tensor("out_sh", [SHARD_PAD, FOUT], f32,
                            kind="ExternalOutput")

    with tile.TileContext(nc) as tc, ExitStack() as ctx:
        consts = ctx.enter_context(tc.tile_pool(name="consts", bufs=1))
        gpool = ctx.enter_context(tc.tile_pool(name="gpool", bufs=2))
        ipool = ctx.enter_context(tc.tile_pool(name="ipool", bufs=2))
        dvpool = ctx.enter_context(tc.tile_pool(name="dvpool", bufs=2))
        spool = ctx.enter_context(tc.tile_pool(name="spool", bufs=8))
        opool = ctx.enter_context(tc.tile_pool(name="opool", bufs=4))
        xpool = ctx.enter_context(tc.tile_pool(name="xpool", bufs=3))
        pmpool = ctx.enter_context(
            tc.tile_pool(name="pmpool", bufs=2, space="PSUM"))
        popool = ctx.enter_context(
            tc.tile_pool(name="popool", bufs=6, space="PSUM"))

        iota_t = consts.tile([P, P], f32)
        nc.sync.dma_start(out=iota_t[:], in_=iota_in[:])
        brep_t = consts.tile([P, FOUT], f32)
        nc.sync.dma_start(out=brep_t[:], in_=b_rep[:])
        w_t = []
        for h in range(KH):
            wt = consts.tile([P, FOUT], f32, tag=f"w{h}")
            nc.sync.dma_start(out=wt[:], in_=w_in[h * P:(h + 1) * P, :])
            w_t.append(wt)

        # ---------------- phase 1: GEMM shard ----------------
        # X^T comes pre-transposed from the host, so lhsT tiles are plain
        # big strided loads (per-partition-contiguous) and PE needs no
        # transposes.  GRP node-columns per load.
        GRP = 1792 if SHARD_PAD % 1792 == 0 else SHARD_PAD
        assert SHARD_PAD % GRP == 0 and GRP % P == 0
        for g in range(SHARD_PAD // GRP):
            xts = []
            for h in range(KH):
                xt = xpool.tile([P, GRP], f32, tag=f"xt{h}")
                nc.sync.dma_start(
                    out=xt[:], in_=xt_sh[h * P:(h + 1) * P,
                                         g * GRP:(g + 1) * GRP])
                xts.append(xt)
            for c in range(GRP // P):
                bk = g * (GRP // P) + c
                mm = pmpool.tile([P, FOUT], f32, tag="mm")
                for h in range(KH):
                    nc.tensor.matmul(
                        out=mm[:], lhsT=xts[h][:, c * P:(c + 1) * P],
                        rhs=w_t[h][:], start=(h == 0), stop=(h == KH - 1))
                om = opool.tile([P, FOUT], f32, tag="om")
                nc.vector.tensor_copy(out=om[:], in_=mm[:])
                nc.sync.dma_start(
                    out=xw_sh[bk * P:(bk + 1) * P, :], in_=om[:])

        # ---------------- phase 2: AllGather the XW table ----------------
        if variant != "p1":
            nc.gpsimd.collective_compute(
                kind="AllGather", op=mybir.AluOpType.bypass,
                replica_groups=[list(range(NCORES))],
                ins=[xw_sh[:]], outs=[table[:]],
            )

        # ---------------- phase 3: gather + segment-sum ----------------
        chunks_bb = st["chunks_bb"]
        sb_b_len = st["sb_b_len"]
        chunk_col0 = st["chunk_col0"]
        idx_col0 = st["idx_col0"]
        blk_chunks = st["blk_chunks"]
        sb_of_blk = st["sb_of_blk"]

        # SWDGE descriptor-ring throttle: chain gather k to gather k-2 so at
        # most ~2 gathers' descriptors are in flight (ring overflow wedges
        # the device otherwise; single_packet must be False for >1024 idxs).
        gather_insts = []

        if variant == "p12":
            ob = opool.tile([P, FOUT], f32, tag="ob")
            nc.sync.dma_start(out=ob[:], in_=table[:P, :])
            nc.sync.dma_start(out=out_sh[:P, :], in_=ob[:])
        if variant == "p1":
            ob = opool.tile([P, FOUT], f32, tag="ob")
            nc.sync.dma_start(out=ob[:], in_=xw_sh[:P, :])
            nc.sync.dma_start(out=out_sh[:P, :], in_=ob[:])

        for sb in (range(NSB) if variant not in ("p12", "p1") else ()):
            blks = [bk for bk in range(NBLK) if sb_of_blk[bk] == sb]
            # super-batch chunk column range (for dest/val)
            c_lo = int(chunk_col0[sb, 0])
            c_hi = (int(chunk_col0[sb + 1, 0]) if sb + 1 < NSB else CTOT)
            dv_w = c_hi - c_lo
            dst_t = dvpool.tile([P, dv_w], f32, tag="dst")
            nc.sync.dma_start(out=dst_t[:], in_=dest_in[:, c_lo:c_hi])
            vl_t = dvpool.tile([P, dv_w], f32, tag="vl")
            nc.sync.dma_start(out=vl_t[:], in_=val_in[:, c_lo:c_hi])

            gts = [None] * NBUCKET
            for b in range(NBUCKET):
                L = int(sb_b_len[sb, b])
                if L == 0:
                    continue
                nch = L // P
                icol = int(idx_col0[sb, b])
                iw = L // 16
                it = ipool.tile([P, iw], i16, tag=f"idx{b}")
                nc.sync.dma_start(out=it[:], in_=idx_in[:, icol:icol + iw])
                gt = gpool.tile([P, nch * FOUT], f32, tag=f"g{b}")
                r_lo = b * WIN
                r_hi = min(r_lo + WIN, TABLE_ROWS)
                gi = nc.gpsimd.dma_gather(
                    out_ap=gt[:].rearrange("p (c f) -> p c f", f=FOUT),
                    in_ap=table[r_lo:r_hi, :],
                    idxs_ap=it[:],
                    num_idxs=L,
                    num_idxs_reg=L,
                    elem_size=FOUT,
                    single_packet=False,
                )
                if len(gather_insts) >= 2:
                    _add_dep_helper(gi.ins, gather_insts[-2], sync=True,
                                    reason="swdge ring throttle")
                gather_insts.append(gi.ins)
                gts[b] = gt

            if variant == "p12g":
                continue

            nb = len(blks)
            ob = opool.tile([P, nb * FOUT], f32, tag="ob")
            for ci, bk in enumerate(blks):
                po = popool.tile([P, FOUT], f32, tag="po")
                lst = blk_chunks[bk]
                for k, (b, j, gcol) in enumerate(lst):
                    s_t = spool.tile([P, P], f32, tag="s")
                    lcol = gcol - c_lo
                    nc.vector.tensor_scalar(
                        out=s_t[:], in0=iota_t[:],
                        scalar1=dst_t[:, lcol:lcol + 1],
                        scalar2=vl_t[:, lcol:lcol + 1],
                        op0=mybir.AluOpType.is_equal,
                        op1=mybir.AluOpType.mult,
                    )
                    nc.tensor.matmul(
                        out=po[:], lhsT=s_t[:],
                        rhs=gts[b][:, j * FOUT:(j + 1) * FOUT],
                        start=(k == 0), stop=(k == len(lst) - 1),
                    )
                nc.vector.tensor_tensor(
                    out=ob[:, ci * FOUT:(ci + 1) * FOUT], in0=po[:],
                    in1=brep_t[:], op=mybir.AluOpType.add)
                nc.vector.tensor_scalar(
                    out=ob[:, ci * FOUT:(ci + 1) * FOUT],
                    in0=ob[:, ci * FOUT:(ci + 1) * FOUT],
                    scalar1=0.0, scalar2=None, op0=mybir.AluOpType.max)
            # one batched store per super-batch: rows [blks[0]*P, ...+nb*P)
            oview = out_sh[blks[0] * P:(blks[0] + nb) * P, :].rearrange(
                "(c p) f -> p c f", p=P)
            nc.sync.dma_start(
                out=oview, in_=ob[:].rearrange("p (c f) -> p c f", f=FOUT))

    nc.compile()
    _split_excess_waits(nc)
    return nc


def _prepare(X, edge_row, edge_col, edge_vals, W, b):
    """Build the compiled Bass program + per-core input maps."""
    X = np.asarray(X, dtype=np.float32)
    edge_row = np.asarray(edge_row, dtype=np.int64)
    edge_col = np.asarray(edge_col, dtype=np.int64)
    edge_vals = np.asarray(edge_vals, dtype=np.float32)
    W = np.asarray(W, dtype=np.float32)
    b = np.asarray(b, dtype=np.float32)

    struct, idx_np, dest_np, val_np = _build_plan(edge_row, edge_col, edge_vals)
    nc = _build_nc(struct)

    b_rep = np.tile(b[None, :], (P, 1)).astype(np.float32)
    iota = np.tile(np.arange(P, dtype=np.float32)[None, :], (P, 1))

    in_maps = []
    for c in range(NCORES):
        xt_pad = np.zeros((FIN, SHARD_PAD), dtype=np.float32)
        xt_pad[:, :SHARD] = X[c * SHARD:(c + 1) * SHARD].T
        in_maps.append({
            "xt_sh": xt_pad, "w_in": W, "b_rep": b_rep,
            "iota_in": iota, "idx_in": idx_np[c], "dest_in": dest_np[c],
            "val_in": val_np[c],
        })
    return nc, in_maps


def _assemble(results):
    return np.concatenate(
        [results[c]["out_sh"][:SHARD] for c in range(NCORES)], axis=0)


def kernel(X, edge_row, edge_col, edge_vals, W, b):
    nc, in_maps = _prepare(X, edge_row, edge_col, edge_vals, W, b)
    res = run_bass_kernel_spmd(nc, in_maps, core_ids=list(range(NCORES)))
    return _assemble(res.results)


# revision 28
# speedup vs baseline: 1.0037x; 1.0037x over previous
"""GCN layer (X@W -> edge gather/scale -> segment-sum by dest -> +b -> relu)
as a Bass/Tile kernel on 8 Trainium2 NeuronCores.

Strategy (1D node partition, SPMD single program):
  - Nodes sharded 12500/core (destination shard).  Each core computes its
    XW shard with PE matmuls, then an AllGather replicates the full XW
    table [100352, 64] f32 into every core's DRAM.
  - Edges partitioned by destination shard, sorted by (dest block of 128,
    source bucket).  Sources are gathered from the table with dma_gather
    (int16 indices -> 4 source windows of 32768 table rows).
  - Per 128-edge chunk a selection matrix S[e, d] = val[e] * (dest[e]==d)
    is built in ONE DVE tensor_scalar op from a constant iota tile, then
    PE computes psum[128 dests, 64] += S^T @ G (gathered rows), giving the
    multiply + segment-sum in one matmul.  +bias and relu on eviction.

All chunk counts are padded to the max over cores so all 8 cores run the
same program (required for the collective / PJRT SPMD launch).
"""

import math
from contextlib import ExitStack

import numpy as np

import concourse.bacc as bacc
import concourse.mybir as mybir
import concourse.tile as tile
from concourse.bass import _add_dep_helper
from concourse.bass_utils import run_bass_kernel_spmd

# Problem constants (hardcoded per contract; kernel.py must be self-contained).
N = 100000
E = 1600000
FIN = 256
FOUT = 64
NCORES = 8

P = 128                      # partitions / block size
SHARD = N // NCORES          # 12500 dest nodes per core
NBLK = math.ceil(SHARD / P)  # 98 dest blocks per core
SHARD_PAD = NBLK * P         # 12544 (X zero-padded rows)
TABLE_ROWS = NCORES * SHARD_PAD  # 100352
WIN = 32768                  # int16-addressable source window (table rows)
NBUCKET = math.ceil(TABLE_ROWS / WIN)  # 4
SB_BLOCKS = 8                # dest blocks per super-batch
NSB = math.ceil(NBLK / SB_BLOCKS)
KH = FIN // P                # 2 contraction halves in the GEMM


def _build_plan(edge_row, edge_col, edge_vals):
    """Host-side edge partition/sort/pad.  Returns the uniform structure
    (shared across cores) + per-core staged arrays."""
    core = edge_row // SHARD
    r_local = edge_row - core * SHARD
    blk = r_local // P
    dest_in_blk = (r_local - blk * P).astype(np.float32)
    src_core = edge_col // SHARD
    table_row = src_core * SHARD_PAD + (edge_col - src_core * SHARD)
    bucket = table_row // WIN
    idx16 = (table_row - bucket * WIN).astype(np.int16)

    # sort edges by (core, blk, bucket)
    order = np.lexsort((bucket, blk, core))
    core_s = core[order]
    blk_s = blk[order]
    bucket_s = bucket[order]
    idx16_s = idx16[order]
    dest_s = dest_in_blk[order]
    val_s = edge_vals[order].astype(np.float32)

    # segment counts per (core, blk, bucket)
    seg_key = (core_s * NBLK + blk_s) * NBUCKET + bucket_s
    counts = np.bincount(seg_key, minlength=NCORES * NBLK * NBUCKET).reshape(
        NCORES, NBLK, NBUCKET
    )
    # uniform capacity (in chunks of 128 edges) per (blk, bucket): max over cores
    chunks_bb = np.ceil(counts / P).astype(np.int64).max(axis=0)  # [NBLK, NBUCKET]
    # guarantee at least one chunk per block overall (needed so PSUM gets reset)
    assert chunks_bb.sum(axis=1).min() >= 1
    cap_bb = chunks_bb * P

    # ---- static layout ----
    # stream order: (sb, bucket, blk in sb, chunk)
    sb_of_blk = np.arange(NBLK) // SB_BLOCKS
    # chunk columns per (sb, bucket): sum of chunks of its blocks
    # slot offsets for each (blk, bucket) within its (sb, bucket) stream
    slot_off = np.zeros((NBLK, NBUCKET), dtype=np.int64)
    sb_b_len = np.zeros((NSB, NBUCKET), dtype=np.int64)   # slots per (sb, bucket)
    for sb in range(NSB):
        blks = np.where(sb_of_blk == sb)[0]
        for b in range(NBUCKET):
            off = 0
            for bk in blks:
                slot_off[bk, b] = off
                off += cap_bb[bk, b]
            sb_b_len[sb, b] = off
    # global offsets: chunk columns and idx columns per (sb, bucket)
    chunk_col0 = np.zeros((NSB, NBUCKET), dtype=np.int64)
    idx_col0 = np.zeros((NSB, NBUCKET), dtype=np.int64)
    ccur = icur = 0
    for sb in range(NSB):
        for b in range(NBUCKET):
            chunk_col0[sb, b] = ccur
            idx_col0[sb, b] = icur
            ccur += sb_b_len[sb, b] // P
            icur += sb_b_len[sb, b] // 16
    CTOT = ccur   # total chunks per core
    ITOT = icur   # total idx columns per core

    # global slot index for every edge:
    #   slot = global_slot0[sb,b] + slot_off[blk,b] + rank_within_segment
    # where global_slot0 = chunk_col0 * 128
    first_of_seg = np.zeros(NCORES * NBLK * NBUCKET + 1, dtype=np.int64)
    np.cumsum(counts.reshape(-1), out=first_of_seg[1:])
    rank = np.arange(len(core_s)) - first_of_seg[seg_key]
    slot = (
        chunk_col0[sb_of_blk[blk_s], bucket_s] * P
        + slot_off[blk_s, bucket_s]
        + rank
    )

    # ---- per-core staged arrays ----
    idx_streams = np.zeros((NCORES, CTOT * P), dtype=np.int16)
    dest_streams = np.zeros((NCORES, CTOT * P), dtype=np.float32)
    val_streams = np.zeros((NCORES, CTOT * P), dtype=np.float32)
    for c in range(NCORES):
        m = core_s == c
        idx_streams[c, slot[m]] = idx16_s[m]
        dest_streams[c, slot[m]] = dest_s[m]
        val_streams[c, slot[m]] = val_s[m]

    # dest/val DRAM layout [128, CTOT]: chunk j, partition p <- stream[j*128+p]
    dest_np = dest_streams.reshape(NCORES, CTOT, P).transpose(0, 2, 1).copy()
    val_np = val_streams.reshape(NCORES, CTOT, P).transpose(0, 2, 1).copy()

    # idx DRAM layout [128, ITOT] int16: within each (sb,b) segment of the
    # stream, idx i -> partition i%16 (replicated over the 8 groups of 16),
    # column i//16
    idx_np = np.zeros((NCORES, P, ITOT), dtype=np.int16)
    for sb in range(NSB):
        for b in range(NBUCKET):
            L = int(sb_b_len[sb, b])
            if L == 0:
                continue
            s0 = int(chunk_col0[sb, b]) * P
            i0 = int(idx_col0[sb, b])
            seg = idx_streams[:, s0:s0 + L].reshape(NCORES, L // 16, 16)
            seg = seg.transpose(0, 2, 1)  # [NCORES, 16, L//16]
            idx_np[:, :, i0:i0 + L // 16] = np.tile(seg, (1, 8, 1))

    # per-block chunk list: (bucket, j_local_in_gather, global_chunk_col)
    blk_chunks = []
    for bk in range(NBLK):
        sb = int(sb_of_blk[bk])
        lst = []
        for b in range(NBUCKET):
            nch = int(chunks_bb[bk, b])
            j0 = int(slot_off[bk, b]) // P
            c0 = int(chunk_col0[sb, b]) + j0
            for k in range(nch):
                lst.append((b, j0 + k, c0 + k))
        blk_chunks.append(lst)

    struct = dict(
        chunks_bb=chunks_bb, sb_b_len=sb_b_len, chunk_col0=chunk_col0,
        idx_col0=idx_col0, CTOT=CTOT, ITOT=ITOT, blk_chunks=blk_chunks,
        sb_of_blk=sb_of_blk,
    )
    return struct, idx_np, dest_np, val_np


_NO_SPLIT = ("InstEventSemaphore", "InstDrain", "InstCollectiveCompute",
             "InstCall", "InstUnconditionalBranch", "InstConditionalBranch")


def _split_excess_waits(nc):
    """Deterministic post-pass: TRN2 instructions tolerate very few sync
    waits (walrus rejects with 'Too many sync wait commands'; Bacc's own
    generate_event_semaphores pass misses cases).  Move all but one
    semaphore wait of every ordinary instruction onto wait-only
    InstEventSemaphore instructions inserted just before it on the same
    engine (engine program order then gates the original instruction)."""
    import concourse.mybir as mybir

    for blk in nc.main_func.blocks:
        out = []
        for ins in blk.instructions:
            si = ins.sync_info
            tn = type(ins).__name__
            if si is None or tn in _NO_SPLIT or len(si.on_wait) <= 1:
                out.append(ins)
                continue
            waits = list(si.on_wait)
            keep, excess = waits[:1], waits[1:]
            while excess:
                batch, excess = excess[:2], excess[2:]
                ev = mybir.InstEventSemaphore(
                    name=nc.get_next_instruction_name(), ins=[], outs=[])
                ev.engine = ins.engine
                ev.sync_info = mybir.SyncInfo(on_wait=batch, on_update=[])
                out.append(ev)
            ins.sync_info = mybir.SyncInfo(
                on_wait=keep, on_update=list(si.on_update))
            out.append(ins)
        blk.instructions[:] = out


def _build_nc(struct, variant="full"):
    # variant: "full" | "p12" (GEMM+collective, dummy out) |
    #          "p12g" (+ gathers, dummy out)
    st = struct
    CTOT, ITOT = st["CTOT"], st["ITOT"]
    nc = bacc.Bacc("TRN2", target_bir_lowering=False, debug=False,
                   num_devices=NCORES)
    f32 = mybir.dt.float32
    i16 = mybir.dt.int16

    xt_sh = nc.dram_tensor("xt_sh", [FIN, SHARD_PAD], f32, kind="ExternalInput")
    w_in = nc.dram_tensor("w_in", [FIN, FOUT], f32, kind="ExternalInput")
    b_rep = nc.dram_tensor("b_rep", [P, FOUT], f32, kind="ExternalInput")
    iota_in = nc.dram_tensor("iota_in", [P, P], f32, kind="ExternalInput")
    idx_in = nc.dram_tensor("idx_in", [P, ITOT], i16, kind="ExternalInput")
    dest_in = nc.dram_tensor("dest_in", [P, CTOT], f32, kind="ExternalInput")
    val_in = nc.dram_tensor("val_in", [P, CTOT], f32, kind="ExternalInput")

    xw_sh = nc.dram_tensor("xw_sh", [SHARD_PAD, FOUT], f32, kind="Internal")
    table = nc.dram_tensor("table", [TABLE_ROWS, FOUT], f32, kind="Internal",
                           addr_space="Shared")
    out_sh = nc.dram_tensor("out_sh", [SHARD_PAD, FOUT], f32,
                            kind="ExternalOutput")

    with tile.TileContext(nc) as tc, ExitStack() as ctx:
        consts = ctx.enter_context(tc.tile_pool(name="consts", bufs=1))
        gpool = ctx.enter_context(tc.tile_pool(name="gpool", bufs=2))
        ipool = ctx.enter_context(tc.tile_pool(name="ipool", bufs=2))
        dvpool = ctx.enter_context(tc.tile_pool(name="dvpool", bufs=2))
        spool = ctx.enter_context(tc.tile_pool(name="spool", bufs=8))
        opool = ctx.enter_context(tc.tile_pool(name="opool", bufs=4))
        xpool = ctx.enter_context(tc.tile_pool(name="xpool", bufs=3))
        pmpool = ctx.enter_context(
            tc.tile_pool(name="pmpool", bufs=2, space="PSUM"))
        popool = ctx.enter_context(
            tc.tile_pool(name="popool", bufs=6, space="PSUM"))

        iota_t = consts.tile([P, P], f32)
        nc.sync.dma_start(out=iota_t[:], in_=iota_in[:])
        brep_t = consts.tile([P, FOUT], f32)
        nc.sync.dma_start(out=brep_t[:], in_=b_rep[:])
        w_t = []
        for h in range(KH):
            wt = consts.tile([P, FOUT], f32, tag=f"w{h}")
            nc.sync.dma_start(out=wt[:], in_=w_in[h * P:(h + 1) * P, :])
            w_t.append(wt)

        # ---------------- phase 1: GEMM shard ----------------
        # X^T comes pre-transposed from the host, so lhsT tiles are plain
        # big strided loads (per-partition-contiguous) and PE needs no
        # transposes.  GRP node-columns per load.
        GRP = 1792 if SHARD_PAD % 1792 == 0 else SHARD_PAD
        assert SHARD_PAD % GRP == 0 and GRP % P == 0
        for g in range(SHARD_PAD // GRP):
            xts = []
            for h in range(KH):
                xt = xpool.tile([P, GRP], f32, tag=f"xt{h}")
                nc.sync.dma_start(
                    out=xt[:], in_=xt_sh[h * P:(h + 1) * P,
                                         g * GRP:(g + 1) * GRP])
                xts.append(xt)
            for c in range(GRP // P):
                bk = g * (GRP // P) + c
                mm = pmpool.tile([P, FOUT], f32, tag="mm")
                for h in range(KH):
                    nc.tensor.matmul(
                        out=mm[:], lhsT=xts[h][:, c * P:(c + 1) * P],
                        rhs=w_t[h][:], start=(h == 0), stop=(h == KH - 1))
                om = opool.tile([P, FOUT], f32, tag="om")
                nc.vector.tensor_copy(out=om[:], in_=mm[:])
                nc.sync.dma_start(
                    out=xw_sh[bk * P:(bk + 1) * P, :], in_=om[:])

        # ---------------- phase 2: AllGather the XW table ----------------
        if variant != "p1":
            nc.gpsimd.collective_compute(
                kind="AllGather", op=mybir.AluOpType.bypass,
                replica_groups=[list(range(NCORES))],
                ins=[xw_sh[:]], outs=[table[:]],
            )

        # ---------------- phase 3: gather + segment-sum ----------------
        chunks_bb = st["chunks_bb"]
        sb_b_len = st["sb_b_len"]
        chunk_col0 = st["chunk_col0"]
        idx_col0 = st["idx_col0"]
        blk_chunks = st["blk_chunks"]
        sb_of_blk = st["sb_of_blk"]

        # SWDGE descriptor-ring throttle: chain gather k to gather k-2 so at
        # most ~2 gathers' descriptors are in flight (ring overflow wedges
        # the device otherwise; single_packet must be False for >1024 idxs).
        gather_insts = []

        if variant == "p12":
            ob = opool.tile([P, FOUT], f32, tag="ob")
            nc.sync.dma_start(out=ob[:], in_=table[:P, :])
            nc.sync.dma_start(out=out_sh[:P, :], in_=ob[:])
        if variant == "p1":
            ob = opool.tile([P, FOUT], f32, tag="ob")
            nc.sync.dma_start(out=ob[:], in_=xw_sh[:P, :])
            nc.sync.dma_start(out=out_sh[:P, :], in_=ob[:])

        for sb in (range(NSB) if variant not in ("p12", "p1") else ()):
            blks = [bk for bk in range(NBLK) if sb_of_blk[bk] == sb]
            # super-batch chunk column range (for dest/val)
            c_lo = int(chunk_col0[sb, 0])
            c_hi = (int(chunk_col0[sb + 1, 0]) if sb + 1 < NSB else CTOT)
            dv_w = c_hi - c_lo
            dst_t = dvpool.tile([P, dv_w], f32, tag="dst")
            nc.sync.dma_start(out=dst_t[:], in_=dest_in[:, c_lo:c_hi])
            vl_t = dvpool.tile([P, dv_w], f32, tag="vl")
            nc.sync.dma_start(out=vl_t[:], in_=val_in[:, c_lo:c_hi])

            gts = [None] * NBUCKET
            for b in range(NBUCKET):
                L = int(sb_b_len[sb, b])
                if L == 0:
                    continue
                nch = L // P
                icol = int(idx_col0[sb, b])
                iw = L // 16
                it = ipool.tile([P, iw], i16, tag=f"idx{b}")
                nc.sync.dma_start(out=it[:], in_=idx_in[:, icol:icol + iw])
                gt = gpool.tile([P, nch * FOUT], f32, tag=f"g{b}")
                r_lo = b * WIN
                r_hi = min(r_lo + WIN, TABLE_ROWS)
                gi = nc.gpsimd.dma_gather(
                    out_ap=gt[:].rearrange("p (c f) -> p c f", f=FOUT),
                    in_ap=table[r_lo:r_hi, :],
                    idxs_ap=it[:],
                    num_idxs=L,
                    num_idxs_reg=L,
                    elem_size=FOUT,
                    single_packet=False,
                )
                if len(gather_insts) >= 2:
                    _add_dep_helper(gi.ins, gather_insts[-2], sync=True,
                                    reason="swdge ring throttle")
                gather_insts.append(gi.ins)
                gts[b] = gt

            if variant == "p12g":
                continue

            for bk in blks:
                po = popool.tile([P, FOUT], f32, tag="po")
                lst = blk_chunks[bk]
                for k, (b, j, gcol) in enumerate(lst):
                    s_t = spool.tile([P, P], f32, tag="s")
                    lcol = gcol - c_lo
                    nc.vector.tensor_scalar(
                        out=s_t[:], in0=iota_t[:],
                        scalar1=dst_t[:, lcol:lcol + 1],
                        scalar2=vl_t[:, lcol:lcol + 1],
                        op0=mybir.AluOpType.is_equal,
                        op1=mybir.AluOpType.mult,
                    )
                    nc.tensor.matmul(
                        out=po[:], lhsT=s_t[:],
                        rhs=gts[b][:, j * FOUT:(j + 1) * FOUT],
                        start=(k == 0), stop=(k == len(lst) - 1),
                    )
                ob = opool.tile([P, FOUT], f32, tag="ob")
                nc.vector.tensor_tensor(
                    out=ob[:], in0=po[:], in1=brep_t[:],
                    op=mybir.AluOpType.add)
                nc.vector.tensor_scalar(
                    out=ob[:], in0=ob[:], scalar1=0.0, scalar2=None,
                    op0=mybir.AluOpType.max)
                nc.sync.dma_start(
                    out=out_sh[bk * P:(bk + 1) * P, :], in_=ob[:])

    nc.compile()
    _split_excess_waits(nc)
    return nc


def _prepare(X, edge_row, edge_col, edge_vals, W, b):
    """Build the compiled Bass program + per-core input maps."""
    X = np.asarray(X, dtype=np.float32)
    edge_row = np.asarray(edge_row, dtype=np.int64)
    edge_col = np.asarray(edge_col, dtype=np.int64)
    edge_vals = np.asarray(edge_vals, dtype=np.float32)
    W = np.asarray(W, dtype=np.float32)
    b = np.asarray(b, dtype=np.float32)

    struct, idx_np, dest_np, val_np = _build_plan(edge_row, edge_col, edge_vals)
    nc = _build_nc(struct)

    b_rep = np.tile(b[None, :], (P, 1)).astype(np.float32)
    iota = np.tile(np.arange(P, dtype=np.float32)[None, :], (P, 1))

    in_maps = []
    for c in range(NCORES):
        xt_pad = np.zeros((FIN, SHARD_PAD), dtype=np.float32)
        xt_pad[:, :SHARD] = X[c * SHARD:(c + 1) * SHARD].T
        in_maps.append({
            "xt_sh": xt_pad, "w_in": W, "b_rep": b_rep,
            "iota_in": iota, "idx_in": idx_np[c], "dest_in": dest_np[c],
            "val_in": val_np[c],
        })
    return nc, in_maps


def _assemble(results):
    return np.concatenate(
        [results[c]["out_sh"][:SHARD] for c in range(NCORES)], axis=0)


def kernel(X, edge_row, edge_col, edge_vals, W, b):
    nc, in_maps = _prepare(X, edge_row, edge_col, edge_vals, W, b)
    res = run_bass_kernel_spmd(nc, in_maps, core_ids=list(range(NCORES)))
    return _assemble(res.results)


# revision 32
# speedup vs baseline: 1.2348x; 1.2303x over previous
"""GCN layer (X@W -> edge gather/scale -> segment-sum by dest -> +b -> relu)
as a Bass/Tile kernel on 8 Trainium2 NeuronCores.

Strategy (1D node partition, SPMD single program):
  - Nodes sharded 12500/core (destination shard).  Each core computes its
    XW shard with PE matmuls, then an AllGather replicates the full XW
    table [100352, 64] f32 into every core's DRAM.
  - Edges partitioned by destination shard, sorted by (dest block of 128,
    source bucket).  Sources are gathered from the table with dma_gather
    (int16 indices -> 4 source windows of 32768 table rows).
  - Per 128-edge chunk a selection matrix S[e, d] = val[e] * (dest[e]==d)
    is built in ONE DVE tensor_scalar op from a constant iota tile, then
    PE computes psum[128 dests, 64] += S^T @ G (gathered rows), giving the
    multiply + segment-sum in one matmul.  +bias and relu on eviction.

All chunk counts are padded to the max over cores so all 8 cores run the
same program (required for the collective / PJRT SPMD launch).
"""

import math
from contextlib import ExitStack

import numpy as np

import concourse.bacc as bacc
import concourse.mybir as mybir
import concourse.tile as tile
from concourse.bass import _add_dep_helper
from concourse.bass_utils import run_bass_kernel_spmd

# Problem constants (hardcoded per contract; kernel.py must be self-contained).
N = 100000
E = 1600000
FIN = 256
FOUT = 64
NCORES = 8

P = 128                      # partitions / block size
SHARD = N // NCORES          # 12500 dest nodes per core
NBLK = math.ceil(SHARD / P)  # 98 dest blocks per core
SHARD_PAD = NBLK * P         # 12544 (X zero-padded rows)
TABLE_ROWS = NCORES * SHARD_PAD  # 100352
WIN = 32768                  # int16-addressable source window (table rows)
NBUCKET = math.ceil(TABLE_ROWS / WIN)  # 4
SB_BLOCKS = 8                # dest blocks per super-batch
NSB = math.ceil(NBLK / SB_BLOCKS)
KH = FIN // P                # 2 contraction halves in the GEMM


def _build_plan(edge_row, edge_col, edge_vals):
    """Host-side edge partition/sort/pad.  Returns the uniform structure
    (shared across cores) + per-core staged arrays."""
    core = edge_row // SHARD
    r_local = edge_row - core * SHARD
    blk = r_local // P
    dest_in_blk = (r_local - blk * P).astype(np.float32)
    src_core = edge_col // SHARD
    table_row = src_core * SHARD_PAD + (edge_col - src_core * SHARD)
    bucket = table_row // WIN
    idx16 = (table_row - bucket * WIN).astype(np.int16)

    # sort edges by (core, blk, bucket)
    order = np.lexsort((bucket, blk, core))
    core_s = core[order]
    blk_s = blk[order]
    bucket_s = bucket[order]
    idx16_s = idx16[order]
    dest_s = dest_in_blk[order]
    val_s = edge_vals[order].astype(np.float32)

    # segment counts per (core, blk, bucket)
    seg_key = (core_s * NBLK + blk_s) * NBUCKET + bucket_s
    counts = np.bincount(seg_key, minlength=NCORES * NBLK * NBUCKET).reshape(
        NCORES, NBLK, NBUCKET
    )
    # uniform capacity (in chunks of 128 edges) per (blk, bucket): max over cores
    chunks_bb = np.ceil(counts / P).astype(np.int64).max(axis=0)  # [NBLK, NBUCKET]
    # guarantee at least one chunk per block overall (needed so PSUM gets reset)
    assert chunks_bb.sum(axis=1).min() >= 1
    cap_bb = chunks_bb * P

    # ---- static layout ----
    # stream order: (sb, bucket, blk in sb, chunk)
    sb_of_blk = np.arange(NBLK) // SB_BLOCKS
    # chunk columns per (sb, bucket): sum of chunks of its blocks
    # slot offsets for each (blk, bucket) within its (sb, bucket) stream
    slot_off = np.zeros((NBLK, NBUCKET), dtype=np.int64)
    sb_b_len = np.zeros((NSB, NBUCKET), dtype=np.int64)   # slots per (sb, bucket)
    for sb in range(NSB):
        blks = np.where(sb_of_blk == sb)[0]
        for b in range(NBUCKET):
            off = 0
            for bk in blks:
                slot_off[bk, b] = off
                off += cap_bb[bk, b]
            sb_b_len[sb, b] = off
    # global offsets: chunk columns and idx columns per (sb, bucket)
    chunk_col0 = np.zeros((NSB, NBUCKET), dtype=np.int64)
    idx_col0 = np.zeros((NSB, NBUCKET), dtype=np.int64)
    ccur = icur = 0
    for sb in range(NSB):
        for b in range(NBUCKET):
            chunk_col0[sb, b] = ccur
            idx_col0[sb, b] = icur
            ccur += sb_b_len[sb, b] // P
            icur += sb_b_len[sb, b] // 16
    CTOT = ccur   # total chunks per core
    ITOT = icur   # total idx columns per core

    # global slot index for every edge:
    #   slot = global_slot0[sb,b] + slot_off[blk,b] + rank_within_segment
    # where global_slot0 = chunk_col0 * 128
    first_of_seg = np.zeros(NCORES * NBLK * NBUCKET + 1, dtype=np.int64)
    np.cumsum(counts.reshape(-1), out=first_of_seg[1:])
    rank = np.arange(len(core_s)) - first_of_seg[seg_key]
    slot = (
        chunk_col0[sb_of_blk[blk_s], bucket_s] * P
        + slot_off[blk_s, bucket_s]
        + rank
    )

    # ---- per-core staged arrays ----
    idx_streams = np.zeros((NCORES, CTOT * P), dtype=np.int16)
    dest_streams = np.zeros((NCORES, CTOT * P), dtype=np.float32)
    val_streams = np.zeros((NCORES, CTOT * P), dtype=np.float32)
    for c in range(NCORES):
        m = core_s == c
        idx_streams[c, slot[m]] = idx16_s[m]
        dest_streams[c, slot[m]] = dest_s[m]
        val_streams[c, slot[m]] = val_s[m]

    # dest/val DRAM layout [128, CTOT]: chunk j, partition p <- stream[j*128+p]
    dest_np = dest_streams.reshape(NCORES, CTOT, P).transpose(0, 2, 1).copy()
    val_np = val_streams.reshape(NCORES, CTOT, P).transpose(0, 2, 1).copy()

    # idx DRAM layout [128, ITOT] int16: within each (sb,b) segment of the
    # stream, idx i -> partition i%16 (replicated over the 8 groups of 16),
    # column i//16
    idx_np = np.zeros((NCORES, P, ITOT), dtype=np.int16)
    for sb in range(NSB):
        for b in range(NBUCKET):
            L = int(sb_b_len[sb, b])
            if L == 0:
                continue
            s0 = int(chunk_col0[sb, b]) * P
            i0 = int(idx_col0[sb, b])
            seg = idx_streams[:, s0:s0 + L].reshape(NCORES, L // 16, 16)
            seg = seg.transpose(0, 2, 1)  # [NCORES, 16, L//16]
            idx_np[:, :, i0:i0 + L // 16] = np.tile(seg, (1, 8, 1))

    # per-block chunk list: (bucket, j_local_in_gather, global_chunk_col)
    blk_chunks = []
    for bk in range(NBLK):
        sb = int(sb_of_blk[bk])
        lst = []
        for b in range(NBUCKET):
            nch = int(chunks_bb[bk, b])
            j0 = int(slot_off[bk, b]) // P
            c0 = int(chunk_col0[sb, b]) + j0
            for k in range(nch):
                lst.append((b, j0 + k, c0 + k))
        blk_chunks.append(lst)

    struct = dict(
        chunks_bb=chunks_bb, sb_b_len=sb_b_len, chunk_col0=chunk_col0,
        idx_col0=idx_col0, CTOT=CTOT, ITOT=ITOT, blk_chunks=blk_chunks,
        sb_of_blk=sb_of_blk,
    )
    return struct, idx_np, dest_np, val_np


_NO_SPLIT = ("InstEventSemaphore", "InstDrain", "InstCollectiveCompute",
             "InstCall", "InstUnconditionalBranch", "InstConditionalBranch")


def _split_excess_waits(nc):
    """Deterministic post-pass: TRN2 instructions tolerate very few sync
    waits (walrus rejects with 'Too many sync wait commands'; Bacc's own
    generate_event_semaphores pass misses cases).  Move all but one
    semaphore wait of every ordinary instruction onto wait-only
    InstEventSemaphore instructions inserted just before it on the same
    engine (engine program order then gates the original instruction)."""
    import concourse.mybir as mybir

    for blk in nc.main_func.blocks:
        out = []
        for ins in blk.instructions:
            si = ins.sync_info
            tn = type(ins).__name__
            if si is None or tn in _NO_SPLIT or len(si.on_wait) <= 1:
                out.append(ins)
                continue
            waits = list(si.on_wait)
            keep, excess = waits[:1], waits[1:]
            while excess:
                batch, excess = excess[:2], excess[2:]
                ev = mybir.InstEventSemaphore(
                    name=nc.get_next_instruction_name(), ins=[], outs=[])
                ev.engine = ins.engine
                ev.sync_info = mybir.SyncInfo(on_wait=batch, on_update=[])
                out.append(ev)
            ins.sync_info = mybir.SyncInfo(
                on_wait=keep, on_update=list(si.on_update))
            out.append(ins)
        blk.instructions[:] = out


def _build_nc(struct, variant="full"):
    # variant: "full" | "p12" (GEMM+collective, dummy out) |
    #          "p12g" (+ gathers, dummy out)
    st = struct
    CTOT, ITOT = st["CTOT"], st["ITOT"]
    nc = bacc.Bacc("TRN2", target_bir_lowering=False, debug=False,
                   num_devices=NCORES)
    f32 = mybir.dt.float32
    i16 = mybir.dt.int16

    xt_sh = nc.dram_tensor("xt_sh", [FIN, SHARD_PAD], f32, kind="ExternalInput")
    w_in = nc.dram_tensor("w_in", [FIN, FOUT], f32, kind="ExternalInput")
    b_rep = nc.dram_tensor("b_rep", [P, FOUT], f32, kind="ExternalInput")
    iota_in = nc.dram_tensor("iota_in", [P, P], f32, kind="ExternalInput")
    idx_in = nc.dram_tensor("idx_in", [P, ITOT], i16, kind="ExternalInput")
    dest_in = nc.dram_tensor("dest_in", [P, CTOT], f32, kind="ExternalInput")
    val_in = nc.dram_tensor("val_in", [P, CTOT], f32, kind="ExternalInput")

    xw_sh = nc.dram_tensor("xw_sh", [SHARD_PAD, FOUT], f32, kind="Internal")
    table = nc.dram_tensor("table", [TABLE_ROWS, FOUT], f32, kind="Internal",
                           addr_space="Shared")
    out_sh = nc.dram_tensor("out_sh", [SHARD_PAD, FOUT], f32,
                            kind="ExternalOutput")

    with tile.TileContext(nc) as tc, ExitStack() as ctx:
        consts = ctx.enter_context(tc.tile_pool(name="consts", bufs=1))
        gpool = ctx.enter_context(tc.tile_pool(name="gpool", bufs=2))
        spool = ctx.enter_context(tc.tile_pool(name="spool", bufs=8))
        opool = ctx.enter_context(tc.tile_pool(name="opool", bufs=4))
        xpool = ctx.enter_context(tc.tile_pool(name="xpool", bufs=3))
        pmpool = ctx.enter_context(
            tc.tile_pool(name="pmpool", bufs=2, space="PSUM"))
        popool = ctx.enter_context(
            tc.tile_pool(name="popool", bufs=6, space="PSUM"))

        iota_t = consts.tile([P, P], f32)
        nc.sync.dma_start(out=iota_t[:], in_=iota_in[:])
        brep_t = consts.tile([P, FOUT], f32)
        nc.sync.dma_start(out=brep_t[:], in_=b_rep[:])
        w_t = []
        for h in range(KH):
            wt = consts.tile([P, FOUT], f32, tag=f"w{h}")
            nc.sync.dma_start(out=wt[:], in_=w_in[h * P:(h + 1) * P, :])
            w_t.append(wt)
        # edge metadata resident in SBUF for the whole kernel (loaded once,
        # so S-builds and gathers never wait on per-super-batch DMAs)
        dst_all = consts.tile([P, CTOT], f32, tag="dstall")
        nc.sync.dma_start(out=dst_all[:], in_=dest_in[:])
        vl_all = consts.tile([P, CTOT], f32, tag="vlall")
        nc.sync.dma_start(out=vl_all[:], in_=val_in[:])
        idx_all = consts.tile([P, ITOT], i16, tag="idxall")
        nc.sync.dma_start(out=idx_all[:], in_=idx_in[:])

        # ---------------- phase 1: GEMM shard ----------------
        # X^T comes pre-transposed from the host, so lhsT tiles are plain
        # big strided loads (per-partition-contiguous) and PE needs no
        # transposes.  GRP node-columns per load.
        GRP = 1792 if SHARD_PAD % 1792 == 0 else SHARD_PAD
        assert SHARD_PAD % GRP == 0 and GRP % P == 0
        for g in range(SHARD_PAD // GRP):
            xts = []
            for h in range(KH):
                xt = xpool.tile([P, GRP], f32, tag=f"xt{h}")
                nc.sync.dma_start(
                    out=xt[:], in_=xt_sh[h * P:(h + 1) * P,
                                         g * GRP:(g + 1) * GRP])
                xts.append(xt)
            for c in range(GRP // P):
                bk = g * (GRP // P) + c
                mm = pmpool.tile([P, FOUT], f32, tag="mm")
                for h in range(KH):
                    nc.tensor.matmul(
                        out=mm[:], lhsT=xts[h][:, c * P:(c + 1) * P],
                        rhs=w_t[h][:], start=(h == 0), stop=(h == KH - 1))
                om = opool.tile([P, FOUT], f32, tag="om")
                nc.vector.tensor_copy(out=om[:], in_=mm[:])
                nc.sync.dma_start(
                    out=xw_sh[bk * P:(bk + 1) * P, :], in_=om[:])

        # ---------------- phase 2: AllGather the XW table ----------------
        if variant != "p1":
            nc.gpsimd.collective_compute(
                kind="AllGather", op=mybir.AluOpType.bypass,
                replica_groups=[list(range(NCORES))],
                ins=[xw_sh[:]], outs=[table[:]],
            )

        # ---------------- phase 3: gather + segment-sum ----------------
        chunks_bb = st["chunks_bb"]
        sb_b_len = st["sb_b_len"]
        chunk_col0 = st["chunk_col0"]
        idx_col0 = st["idx_col0"]
        blk_chunks = st["blk_chunks"]
        sb_of_blk = st["sb_of_blk"]

        # SWDGE descriptor-ring throttle: chain gather k to gather k-2 so at
        # most ~2 gathers' descriptors are in flight (ring overflow wedges
        # the device otherwise; single_packet must be False for >1024 idxs).
        gather_insts = []

        if variant == "p12":
            ob = opool.tile([P, FOUT], f32, tag="ob")
            nc.sync.dma_start(out=ob[:], in_=table[:P, :])
            nc.sync.dma_start(out=out_sh[:P, :], in_=ob[:])
        if variant == "p1":
            ob = opool.tile([P, FOUT], f32, tag="ob")
            nc.sync.dma_start(out=ob[:], in_=xw_sh[:P, :])
            nc.sync.dma_start(out=out_sh[:P, :], in_=ob[:])

        for sb in (range(NSB) if variant not in ("p12", "p1") else ()):
            blks = [bk for bk in range(NBLK) if sb_of_blk[bk] == sb]

            gts = [None] * NBUCKET
            for b in range(NBUCKET):
                L = int(sb_b_len[sb, b])
                if L == 0:
                    continue
                nch = L // P
                icol = int(idx_col0[sb, b])
                iw = L // 16
                gt = gpool.tile([P, nch * FOUT], f32, tag=f"g{b}")
                r_lo = b * WIN
                r_hi = min(r_lo + WIN, TABLE_ROWS)
                gi = nc.gpsimd.dma_gather(
                    out_ap=gt[:].rearrange("p (c f) -> p c f", f=FOUT),
                    in_ap=table[r_lo:r_hi, :],
                    idxs_ap=idx_all[:, icol:icol + iw],
                    num_idxs=L,
                    num_idxs_reg=L,
                    elem_size=FOUT,
                    single_packet=False,
                )
                if len(gather_insts) >= 2:
                    _add_dep_helper(gi.ins, gather_insts[-2], sync=True,
                                    reason="swdge ring throttle")
                gather_insts.append(gi.ins)
                gts[b] = gt

            if variant == "p12g":
                continue

            for bk in blks:
                po = popool.tile([P, FOUT], f32, tag="po")
                lst = blk_chunks[bk]
                for k, (b, j, gcol) in enumerate(lst):
                    s_t = spool.tile([P, P], f32, tag="s")
                    nc.vector.tensor_scalar(
                        out=s_t[:], in0=iota_t[:],
                        scalar1=dst_all[:, gcol:gcol + 1],
                        scalar2=vl_all[:, gcol:gcol + 1],
                        op0=mybir.AluOpType.is_equal,
                        op1=mybir.AluOpType.mult,
                    )
                    nc.tensor.matmul(
                        out=po[:], lhsT=s_t[:],
                        rhs=gts[b][:, j * FOUT:(j + 1) * FOUT],
                        start=(k == 0), stop=(k == len(lst) - 1),
                    )
                ob = opool.tile([P, FOUT], f32, tag="ob")
                nc.vector.tensor_tensor(
                    out=ob[:], in0=po[:], in1=brep_t[:],
                    op=mybir.AluOpType.add)
                nc.vector.tensor_scalar(
                    out=ob[:], in0=ob[:], scalar1=0.0, scalar2=None,
                    op0=mybir.AluOpType.max)
                nc.sync.dma_start(
                    out=out_sh[bk * P:(bk + 1) * P, :], in_=ob[:])

    nc.compile()
    _split_excess_waits(nc)
    return nc


def _prepare(X, edge_row, edge_col, edge_vals, W, b):
    """Build the compiled Bass program + per-core input maps."""
    X = np.asarray(X, dtype=np.float32)
    edge_row = np.asarray(edge_row, dtype=np.int64)
    edge_col = np.asarray(edge_col, dtype=np.int64)
    edge_vals = np.asarray(edge_vals, dtype=np.float32)
    W = np.asarray(W, dtype=np.float32)
    b = np.asarray(b, dtype=np.float32)

    struct, idx_np, dest_np, val_np = _build_plan(edge_row, edge_col, edge_vals)
    nc = _build_nc(struct)

    b_rep = np.tile(b[None, :], (P, 1)).astype(np.float32)
    iota = np.tile(np.arange(P, dtype=np.float32)[None, :], (P, 1))

    in_maps = []
    for c in range(NCORES):
        xt_pad = np.zeros((FIN, SHARD_PAD), dtype=np.float32)
        xt_pad[:, :SHARD] = X[c * SHARD:(c + 1) * SHARD].T
        in_maps.append({
            "xt_sh": xt_pad, "w_in": W, "b_rep": b_rep,
            "iota_in": iota, "idx_in": idx_np[c], "dest_in": dest_np[c],
            "val_in": val_np[c],
        })
    return nc, in_maps


def _assemble(results):
    return np.concatenate(
        [results[c]["out_sh"][:SHARD] for c in range(NCORES)], axis=0)


def kernel(X, edge_row, edge_col, edge_vals, W, b):
    nc, in_maps = _prepare(X, edge_row, edge_col, edge_vals, W, b)
    res = run_bass_kernel_spmd(nc, in_maps, core_ids=list(range(NCORES)))
    return _assemble(res.results)
